# revision 14
# baseline (speedup 1.0000x reference)
"""Trainium2 Bass kernel for nn_DetectionLoss (YOLO-style detection loss).

Structure:
  * Device (8 NeuronCores, batch sharded 2 images/core, SPMD): streams the two
    large classification-logit tensors once and computes, per (image, branch),
    the target-independent part of the BCE loss sum(softplus(x)) with a
    two-path split that balances the Activation and Vector engines:

      Path A (~80% of columns, shipped as fp8_e4m3): softplus(x) =
        -ln(sigmoid(-x)). One full-rate ACT pass computes sigmoid(-x) (free
        scale=-1, fp8 input) into bf16; the DVE then takes a 5-level halving
        product tree (chunks of 32); the log of each chunk product is
        recovered WITHOUT another ACT pass from the bf16 bit pattern:
        log2 w = (int16view(w) - 16256)/128 + eps(mantissa) with
        E[eps] = 1.5 - 1/ln2, so a single int16 tensor_reduce(add) per chunk
        group yields sum(log2 w) up to a host-side affine fixup.  Only one
        activation table set (sigmoid) is ever loaded.

      Path B (~20% of columns, shipped as fp16, DVE-only): u = e^x via the
        int16 bit-trick (tensor_scalar mult+add -> int16, reinterpreted as
        fp16: piecewise-linear 2^t with a centered magic bias), v = 1 + u,
        the same bf16 product tree and bitcast reduce (ln prod(1+e^x)).
        This offloads ~29% of the ACT work onto spare DVE cycles; the B
        region is processed as one merged chain (all 4 units) with a
        host-side column interleave that keeps every strided chunk within
        one (image, branch) unit.

    fp8/fp16 quantization and the bit-trick PWL errors are zero-mean by
    construction (centered bias constants); measured end-to-end rel err of
    the summed losses is ~2e-5 against the fp32 reference (tolerance 2e-2).

  * Host (numpy, sparse): the TaskAligned assignment only ever involves
    anchors whose center lies inside a gt box (align==0 elsewhere), so the
    DFL box decode, topk/argmax assignment, and the fg-masked loss terms (box
    CIoU, DFL cross-entropy, BCE fg correction) are assembled from
    O(candidates) gathers in exact f32 - the box_regs tensors are only ever
    consumed at those sparse anchors, so decoding them densely on device
    would be wasted work. Mirrors the reference's f32 semantics exactly
    (including jax.lax.top_k's lowest-index tie fill among zero-align
    anchors).
"""
import numpy as np
from contextlib import ExitStack
import ml_dtypes

B, M, NCLS, RM = 16, 32, 80, 16
N = 8400
NCORES = 8
NUNITS = 4                      # 2 local images x 2 branches per core
UCR = N * NCLS // 128           # 5250 real cols per unit
EPS = np.float32(1e-7)
F32 = np.float32
LN2 = float(np.log(2.0))
EPS_MEAN = 1.5 - 1.0 / LN2      # E[log2(1+t)-t], t~U[0,1)
LOG2E = float(np.log2(np.e))
EXP_C1 = 1024.0 * LOG2E         # fp16 bit-trick exp: i16 = x*C1 + C2
EXP_C2 = 15360.0 - 58.68        # centered magic bias
PAD_A = -20.0                   # sigmoid(20) -> 1.0 exactly in bf16 (neutral)
PAD_B = -10.0                   # bit-exp -> ~3e-5 denormal, ln1p ~ 0

CA = 3232                       # path-A (fp8) cols per unit (div 32)
CB = 2048                       # path-B (fp16) cols per unit (div 32)
TBB = 4 * CB                    # merged B region (all 4 units)
GB = CB // 32                   # B chunk-32 count per unit
SIGMA_CHUNKS = {                # per-unit sigma instruction split (div 32)
    0: [1056, 2176],
    1: [3232],
    2: [3232],
    3: [2176, 1056],
}

_CACHE = {}
LAST_RESULT = None          # BassKernelResults of the most recent run


# --------------------------------------------------------------------------
# device program
# --------------------------------------------------------------------------

def _tree(nc, mybir, pools, src_ap, cols, tag):
    """5-level halving product tree; returns w tile [128, cols//32] bf16.
    Chunk-32 products stay in bf16 range for this data: path-A products
    shrink (sigma<=1, min ~e^-60 vs underflow at e^-87); path-B (1+e^x)
    products grow to ~e^26 typical / ~e^45 tail vs overflow at e^88."""
    BF = mybir.dt.bfloat16
    r1p, r2p = pools
    c2, c4, c8, c16, c32 = (cols // 2, cols // 4, cols // 8, cols // 16,
                            cols // 32)
    r1 = r1p.tile([128, c2], BF, tag=f"r1_{c2}", name=f"r1{tag}")
    nc.vector.tensor_mul(r1[:, :], src_ap[:, :c2], src_ap[:, c2:cols])
    r2 = r2p.tile([128, c4], BF, tag=f"r2_{c4}", name=f"r2{tag}")
    nc.vector.tensor_mul(r2[:, :], r1[:, :c4], r1[:, c4:])
    r3 = r1p.tile([128, c8], BF, tag=f"r3_{c8}", name=f"r3{tag}")
    nc.vector.tensor_mul(r3[:, :], r2[:, :c8], r2[:, c8:])
    r4 = r2p.tile([128, c16], BF, tag=f"r4_{c16}", name=f"r4{tag}")
    nc.vector.tensor_mul(r4[:, :], r3[:, :c16], r3[:, c16:])
    w = r1p.tile([128, c32], BF, tag=f"w_{c32}", name=f"w{tag}")
    nc.vector.tensor_mul(w[:, :], r4[:, :c32], r4[:, c32:])
    return w


def _build_program(reps=1, staggered=True):
    import concourse.bacc as bacc
    import concourse.tile as tile
    import concourse.mybir as mybir

    FD = mybir.dt.float32
    FH = mybir.dt.float16
    BF = mybir.dt.bfloat16
    F8 = mybir.dt.float8e4
    I16 = mybir.dt.int16
    AF = mybir.ActivationFunctionType
    AL = mybir.AluOpType
    AX = mybir.AxisListType

    n_chunks = sum(len(SIGMA_CHUNKS[u]) for u in range(NUNITS))
    n_acc = n_chunks + NUNITS   # A columns then B columns

    nc = bacc.Bacc("TRN2", target_bir_lowering=False, debug=False,
                   enable_asserts=False, num_devices=NCORES)
    clsx8 = nc.dram_tensor("clsx8", [128, NUNITS * CA], F8,
                           kind="ExternalInput").ap()
    clsx16 = nc.dram_tensor("clsx16", [128, TBB], FH,
                            kind="ExternalInput").ap()
    acc = nc.dram_tensor("acc", [128, n_acc], FD, kind="ExternalOutput").ap()

    with tile.TileContext(nc) as tc, ExitStack() as ctx:
        io8 = ctx.enter_context(tc.tile_pool(name="io8", bufs=2))
        io16 = ctx.enter_context(tc.tile_pool(name="io16", bufs=2))
        sgp = ctx.enter_context(tc.tile_pool(name="sgp", bufs=2))
        bvp = ctx.enter_context(tc.tile_pool(name="bvp", bufs=2))
        r1p = ctx.enter_context(tc.tile_pool(name="r1p", bufs=2))
        r2p = ctx.enter_context(tc.tile_pool(name="r2p", bufs=2))
        accp = ctx.enter_context(tc.tile_pool(name="accp", bufs=2))
        pools = (r1p, r2p)

        if reps > 1 and staggered:
            tc.prologue_barrier()
        rep_ctx = (tc.For_i(0, reps, 1, staggered_reset=staggered)
                   if reps > 1 else None)
        if rep_ctx is not None:
            rep_ctx.__enter__()

        acc_t = accp.tile([128, n_acc], FD, tag="acc")

        a_tiles = {}
        def dma_unit(u):
            off = 0
            for ci, cc in enumerate(SIGMA_CHUNKS[u]):
                t = io8.tile([128, cc], F8, tag=f"in8_{cc}_{ci}",
                             name=f"in8_{u}_{ci}")
                nc.sync.dma_start(
                    out=t[:], in_=clsx8[:, u * CA + off:u * CA + off + cc])
                a_tiles[(u, ci)] = (t, cc)
                off += cc

        tbb = io16.tile([128, TBB], FH, tag="in16", name="tbb")
        # DMA order: B pieces interleaved between A units so neither path's
        # first compute is delayed.
        dma_unit(0)
        nc.sync.dma_start(out=tbb[:, :CB], in_=clsx16[:, :CB])
        dma_unit(1)
        nc.sync.dma_start(out=tbb[:, CB:2 * CB], in_=clsx16[:, CB:2 * CB])
        dma_unit(2)
        nc.sync.dma_start(out=tbb[:, 2 * CB:3 * CB],
                          in_=clsx16[:, 2 * CB:3 * CB])
        dma_unit(3)
        nc.sync.dma_start(out=tbb[:, 3 * CB:], in_=clsx16[:, 3 * CB:])

        # ACT queue: all sigma chunks back-to-back
        sg_tiles = {}
        for u in range(NUNITS):
            for ci in range(len(SIGMA_CHUNKS[u])):
                t, cc = a_tiles[(u, ci)]
                sg = sgp.tile([128, cc], BF, tag=f"sg_{cc}_{ci}",
                              name=f"sg_{u}_{ci}")
                nc.scalar.activation(sg[:], t[:], AF.Sigmoid, scale=-1.0)
                sg_tiles[(u, ci)] = (sg, cc)

        # DVE queue
        ai = 0
        def emit_a(u, ci):
            nonlocal ai
            sg, cc = sg_tiles[(u, ci)]
            w = _tree(nc, mybir, pools, sg, cc, f"a{u}_{ci}")
            nc.vector.tensor_reduce(acc_t[:, ai:ai + 1], w[:].bitcast(I16),
                                    AX.X, AL.add)
            ai += 1

        def emit_b():
            # bufs=1: ui is consumed by the immediately-following add1 on the
            # same engine queue, so a single buffer costs no overlap.
            ui = bvp.tile([128, TBB], I16, tag="ui", name="ui", bufs=1)
            nc.vector.tensor_scalar(ui[:], tbb[:], EXP_C1, EXP_C2,
                                    AL.mult, AL.add)
            v = bvp.tile([128, TBB], FH, tag="v", name="v")
            nc.vector.tensor_scalar_add(v[:], ui[:].bitcast(FH), 1.0)
            wb = _tree(nc, mybir, pools, v, TBB, "b")
            for u in range(NUNITS):
                nc.vector.tensor_reduce(
                    acc_t[:, n_chunks + u:n_chunks + u + 1],
                    wb[:, u * GB:(u + 1) * GB].bitcast(I16), AX.X, AL.add)

        for u in range(NUNITS):
            for ci in range(len(SIGMA_CHUNKS[u])):
                emit_a(u, ci)
            if u == 1:
                emit_b()

        nc.sync.dma_start(out=acc[:, :], in_=acc_t[:])

        if rep_ctx is not None:
            rep_ctx.__exit__(None, None, None)
            if staggered:
                tc.epilogue_barrier()

    nc.compile()
    return nc


def _make_in_maps(inputs):
    """Per-core inputs: clsx8 [128, 4*CA] fp8 (path A) and clsx16
    [128, TBB] fp16 (path B, chunk-interleaved). Unit u = il*2+br for the
    core's il-th local image and branch br (0=one2many cls, 1=one2one)."""
    cls_b = [np.asarray(inputs["cls_scores"]), np.asarray(inputs["one2one_cls"])]
    cls_r = [np.ascontiguousarray(c).reshape(B, 128, UCR) for c in cls_b]
    stride = TBB // 32      # member j of chunk c sits at col c + j*stride
    in_maps = []
    for i in range(NCORES):
        m8 = np.full((128, NUNITS * CA), PAD_A, ml_dtypes.float8_e4m3)
        m16 = np.full((128, TBB), PAD_B, np.float16)
        for il in range(2):
            b = 2 * i + il
            for br in range(2):
                u = il * 2 + br
                r = cls_r[br][b]
                m8[:, u * CA:(u + 1) * CA] = r[:, :CA].astype(
                    ml_dtypes.float8_e4m3)
                src = np.full((128, CB), PAD_B, np.float16)
                src[:, :UCR - CA] = r[:, CA:].astype(np.float16)
                sv = src.reshape(128, GB, 32)
                base = u * GB
                for j in range(32):
                    m16[:, base + j * stride:base + j * stride + GB] = sv[:, :, j]
        in_maps.append({"clsx8": m8, "clsx16": m16})
    return in_maps


def _bce_from_results(results):
    """Recover bce[b, br] = sum softplus(logits) from the per-core int16
    bit-view sums via the affine log2 fixup."""
    n_chunks = sum(len(SIGMA_CHUNKS[u]) for u in range(NUNITS))
    a_cols = []
    col = 0
    for u in range(NUNITS):
        for ci in range(len(SIGMA_CHUNKS[u])):
            a_cols.append((col, u, SIGMA_CHUNKS[u][ci] // 32))
            col += 1
    bce = np.zeros((B, 2), np.float64)
    for i in range(NCORES):
        acc = results[i]["acc"].astype(np.float64)
        for il in range(2):
            b = 2 * i + il
            for br in range(2):
                u = il * 2 + br
                tot = 0.0
                for (c, cu, g) in a_cols:
                    if cu != u:
                        continue
                    n = 128 * g
                    S = acc[:, c].sum()
                    tot -= LN2 * ((S - n * 16256.0) / 128.0 + n * EPS_MEAN)
                nB = 128 * GB
                SB = acc[:, n_chunks + u].sum()
                tot += LN2 * ((SB - nB * 16256.0) / 128.0 + nB * EPS_MEAN)
                bce[b, br] = tot
    return bce


# --------------------------------------------------------------------------
# host-side sparse decode + assignment + loss assembly (exact f32)
# --------------------------------------------------------------------------

def _sigmoid_f32(x):
    x = x.astype(np.float32)
    out = np.empty_like(x)
    pos = x >= 0
    out[pos] = F32(1.0) / (F32(1.0) + np.exp(-x[pos]))
    ex = np.exp(x[~pos])
    out[~pos] = ex / (F32(1.0) + ex)
    return out


def _host_losses(inputs, bce_const):
    """bce_const: (B,2) float64 sums of softplus(cls logits) from the device."""
    anchors = np.asarray(inputs["anchors"], np.float32)
    strides = np.asarray(inputs["strides_tensor"], np.float32)
    gt_bboxes = np.asarray(inputs["gt_bboxes"], np.float32)
    gt_labels = np.asarray(inputs["gt_labels"])[..., 0].astype(np.int64)
    mask_gt = np.asarray(inputs["mask_gt"])[..., 0].astype(np.float32)
    ax, ay = anchors[:, 0], anchors[:, 1]
    proj = np.arange(RM, dtype=np.float32)

    branch_cls = [np.asarray(inputs["cls_scores"]), np.asarray(inputs["one2one_cls"])]
    branch_reg = [np.asarray(inputs["box_regs"]), np.asarray(inputs["one2one_reg"])]
    branch_topk = [10, 1]

    totals = []
    for br in range(2):
        topk = branch_topk[br]
        n_pos = 0
        xt_sum = np.float64(0.0)
        box_sum = np.float64(0.0)
        dfl_sum = np.float64(0.0)
        bce_sum = np.float64(0.0)
        for b in range(B):
            gt = gt_bboxes[b]
            lab = gt_labels[b]
            mg = mask_gt[b]
            cls_b = branch_cls[br][b]
            reg_b = branch_reg[br][b]
            bce_sum += np.float64(bce_const[b, br])

            # candidate pairs: anchor center inside gt box (align==0 elsewhere)
            ing = ((ax[None, :] >= gt[:, 0:1]) & (ax[None, :] <= gt[:, 2:3])
                   & (ay[None, :] >= gt[:, 1:2]) & (ay[None, :] <= gt[:, 3:4]))
            mi_p, ni_p = np.nonzero(ing)

            # sparse DFL decode at the unique candidate anchors (exact f32,
            # matching jax.nn.softmax's max-subtracted semantics)
            uniq, inv = np.unique(ni_p, return_inverse=True)
            X = reg_b[uniq].astype(np.float32).reshape(-1, 4, RM)
            Xm = X.max(-1, keepdims=True)
            E = np.exp(X - Xm)
            SM = E / E.sum(-1, keepdims=True)
            d = (SM * proj).sum(-1)  # (U,4)
            au = anchors[uniq]
            su = strides[uniq][:, None]
            pd_u = np.concatenate([au - d[:, :2] * su, au + d[:, 2:] * su], -1)
            pa_u = (pd_u[:, 2] - pd_u[:, 0]) * (pd_u[:, 3] - pd_u[:, 1])

            pdp = pd_u[inv]
            gtp = gt[mi_p]
            lt = np.maximum(pdp[:, :2], gtp[:, :2])
            rb = np.minimum(pdp[:, 2:], gtp[:, 2:])
            whp = np.clip(rb - lt, F32(0.0), None)
            inter = whp[:, 0] * whp[:, 1]
            ga = (gt[:, 2] - gt[:, 0]) * (gt[:, 3] - gt[:, 1])
            union = pa_u[inv] + ga[mi_p] - inter + EPS
            iou_p = inter / union
            sig_p = _sigmoid_f32(cls_b[ni_p, lab[mi_p]])
            align_p = sig_p * np.power(iou_p, F32(6.0))

            # topk per gt with jax.lax.top_k tie semantics (stable, then
            # lowest-index zero-align fill when fewer than topk positives)
            sel = [None] * M
            for m in range(M):
                if mg[m] == 0.0:
                    continue
                pm = mi_p == m
                nn = ni_p[pm]
                vv = align_p[pm]
                posm = vv > 0
                npos_m = int(posm.sum())
                if npos_m >= topk:
                    o = np.argsort(-vv, kind="stable")[:topk]
                    sel[m] = set(nn[o].tolist())
                else:
                    s = set(nn[posm].tolist())
                    nfill = topk - npos_m
                    fill = []
                    pos_sorted = np.sort(nn[posm])
                    pi = 0
                    cand = 0
                    while len(fill) < nfill:
                        while pi < len(pos_sorted) and pos_sorted[pi] < cand:
                            pi += 1
                        if pi < len(pos_sorted) and pos_sorted[pi] == cand:
                            pi += 1
                        else:
                            fill.append(cand)
                        cand += 1
                    sel[m] = s | set(fill)

            # argmax over gts per anchor (first index on ties; zeros -> 0)
            colmax = np.zeros(N, np.float32)
            np.maximum.at(colmax, ni_p, align_p)
            mi_arr = np.zeros(N, np.int64)
            has = colmax > 0
            best = np.full(N, 1 << 30, np.int64)
            hit = align_p == colmax[ni_p]
            np.minimum.at(best, ni_p[hit], mi_p[hit])
            mi_arr[has] = best[has]

            fg = np.zeros(N, bool)
            for m in range(M):
                if not sel[m]:
                    continue
                idxs = np.fromiter(sel[m], dtype=np.int64)
                fg[idxs[mi_arr[idxs] == m]] = True
            tgi = np.where(fg, mi_arr, 0)
            n_pos += int(fg.sum())

            idx = np.nonzero(fg)[0]
            if idx.size:
                tb = gt[tgi[idx]]
                pb = pd_u[np.searchsorted(uniq, idx)]
                iw = np.clip(np.minimum(pb[:, 2], tb[:, 2]) - np.maximum(pb[:, 0], tb[:, 0]),
                             F32(0.0), None)
                ih = np.clip(np.minimum(pb[:, 3], tb[:, 3]) - np.maximum(pb[:, 1], tb[:, 1]),
                             F32(0.0), None)
                inter2 = iw * ih
                w1 = pb[:, 2] - pb[:, 0]
                h1 = pb[:, 3] - pb[:, 1]
                w2 = tb[:, 2] - tb[:, 0]
                h2 = tb[:, 3] - tb[:, 1]
                un2 = w1 * h1 + w2 * h2 - inter2 + EPS
                iou2 = inter2 / un2
                xg = cls_b[idx, lab[tgi[idx]]]
                xt_sum += np.float64((xg.astype(np.float64) * iou2.astype(np.float64)).sum())
                # ciou, replicating the reference's min(b1y1, b1y1) quirk
                cw = np.maximum(pb[:, 2], tb[:, 2]) - np.minimum(pb[:, 0], tb[:, 0])
                ch = np.maximum(pb[:, 3], tb[:, 3]) - np.minimum(pb[:, 1], pb[:, 1])
                c2 = cw * cw + ch * ch + EPS
                rho2 = ((pb[:, 0] + pb[:, 2] - tb[:, 0] - tb[:, 2]) ** 2
                        + (pb[:, 1] + pb[:, 3] - tb[:, 1] - tb[:, 3]) ** 2) / F32(4.0)
                v = (F32(4.0) / F32(np.pi) ** 2) * (
                    np.arctan(w2 / (h2 + EPS)) - np.arctan(w1 / (h1 + EPS))) ** 2
                alpha = v / (v - iou2 + (F32(1.0) + EPS))
                ciou = iou2 - (rho2 / c2 + v * alpha)
                box_sum += np.float64((F32(1.0) - ciou).astype(np.float64).sum())
                # dfl
                s = strides[idx]
                a = anchors[idx]
                ltd = (a - tb[:, :2]) / s[:, None]
                rbd = (tb[:, 2:] - a) / s[:, None]
                t4 = np.clip(np.concatenate([ltd, rbd], -1), F32(0.0), F32(RM - 1.01))
                tl = t4.astype(np.int32)
                tr = tl + 1
                wl = tr.astype(np.float32) - t4
                wr = F32(1.0) - wl
                Xd = reg_b[idx].reshape(-1, 4, RM).astype(np.float32)
                mx = Xd.max(-1, keepdims=True)
                lse = np.log(np.exp(Xd - mx).sum(-1, keepdims=True)) + mx
                logp = Xd - lse
                gl = np.take_along_axis(logp, tl[..., None], -1)[..., 0]
                gr = np.take_along_axis(logp, tr[..., None], -1)[..., 0]
                dfl_sum += np.float64((-(gl * wl + gr * wr)).astype(np.float64).sum())

        n_fg = max(float(n_pos), 1.0)
        loss_cls = (bce_sum - xt_sum) / n_fg
        loss_box = box_sum / n_fg
        loss_dfl = dfl_sum / (n_fg * 4.0)
        total = loss_cls * 1.0 + loss_box * 7.5 + loss_dfl * 1.5
        totals.append((total, loss_cls, loss_box, loss_dfl))

    t1, c1, b1, d1 = totals[0]
    t2, c2, b2, d2 = totals[1]
    return np.array([t1 + t2, c1 + c2, b1 + b2, d1 + d2, t1, t2], np.float32)


# --------------------------------------------------------------------------
# entry point
# --------------------------------------------------------------------------

def kernel(**inputs):
    global LAST_RESULT
    from concourse.bass_utils import run_bass_kernel_spmd

    nc = _CACHE.get("nc")
    if nc is None:
        nc = _build_program()
        _CACHE["nc"] = nc

    in_maps = _make_in_maps(inputs)
    res = run_bass_kernel_spmd(nc, in_maps, list(range(NCORES)))
    LAST_RESULT = res

    bce = _bce_from_results(res.results)
    return _host_losses(inputs, bce)


# revision 20
# speedup vs baseline: 1.0119x; 1.0119x over previous
"""Trainium2 Bass kernel for nn_DetectionLoss (YOLO-style detection loss).

Structure:
  * Device (8 NeuronCores, batch sharded 2 images/core, SPMD): streams the two
    large classification-logit tensors once and computes, per (image, branch),
    the target-independent part of the BCE loss sum(softplus(x)) with a
    two-path split that balances the Activation and Vector engines:

      Path A (~80% of columns, shipped as fp8_e4m3): softplus(x) =
        -ln(sigmoid(-x)). One full-rate ACT pass computes sigmoid(-x) (free
        scale=-1, fp8 input) into bf16; the DVE then takes a 5-level halving
        product tree (chunks of 32); the log of each chunk product is
        recovered WITHOUT another ACT pass from the bf16 bit pattern:
        log2 w = (int16view(w) - 16256)/128 + eps(mantissa) with
        E[eps] = 1.5 - 1/ln2, so a single int16 tensor_reduce(add) per chunk
        group yields sum(log2 w) up to a host-side affine fixup.  Only one
        activation table set (sigmoid) is ever loaded.

      Path B (~20% of columns, shipped as fp16, DVE-only): u = e^x via the
        int16 bit-trick (tensor_scalar mult+add -> int16, reinterpreted as
        fp16: piecewise-linear 2^t with a centered magic bias), v = 1 + u,
        the same bf16 product tree and bitcast reduce (ln prod(1+e^x)).
        This offloads ~29% of the ACT work onto spare DVE cycles; the B
        region is processed as one merged chain (all 4 units) with a
        host-side column interleave that keeps every strided chunk within
        one (image, branch) unit.

    fp8/fp16 quantization and the bit-trick PWL errors are zero-mean by
    construction (centered bias constants); measured end-to-end rel err of
    the summed losses is ~2e-5 against the fp32 reference (tolerance 2e-2).

  * Host (numpy, sparse): the TaskAligned assignment only ever involves
    anchors whose center lies inside a gt box (align==0 elsewhere), so the
    DFL box decode, topk/argmax assignment, and the fg-masked loss terms (box
    CIoU, DFL cross-entropy, BCE fg correction) are assembled from
    O(candidates) gathers in exact f32 - the box_regs tensors are only ever
    consumed at those sparse anchors, so decoding them densely on device
    would be wasted work. Mirrors the reference's f32 semantics exactly
    (including jax.lax.top_k's lowest-index tie fill among zero-align
    anchors).
"""
import numpy as np
from contextlib import ExitStack
import ml_dtypes

B, M, NCLS, RM = 16, 32, 80, 16
N = 8400
NCORES = 8
NUNITS = 4                      # 2 local images x 2 branches per core
UCR = N * NCLS // 128           # 5250 real cols per unit
EPS = np.float32(1e-7)
F32 = np.float32
LN2 = float(np.log(2.0))
EPS_MEAN = 1.5 - 1.0 / LN2      # E[log2(1+t)-t], t~U[0,1)
LOG2E = float(np.log2(np.e))
EXP_C1 = 1024.0 * LOG2E         # fp16 bit-trick exp: i16 = x*C1 + C2
EXP_C2 = 15360.0 - 58.68        # centered magic bias
PAD_A = -20.0                   # sigmoid(20) -> 1.0 exactly in bf16 (neutral)
PAD_B = -10.0                   # bit-exp -> ~3e-5 denormal, ln1p ~ 0

CA = 3744                       # path-A (fp8) cols per unit (div 32)
CB = 1536                       # path-B (fp16) cols per unit (div 32)
TBB = 4 * CB                    # merged B region (all 4 units)
GB = CB // 32                   # B chunk-32 count per unit
SIGMA_CHUNKS = {                # per-unit sigma instruction split (div 32)
    0: [1248, 2496],
    1: [3744],
    2: [3744],
    3: [2496, 1248],
}

_CACHE = {}
LAST_RESULT = None          # BassKernelResults of the most recent run


# --------------------------------------------------------------------------
# device program
# --------------------------------------------------------------------------

def _tree(nc, mybir, pools, src_ap, cols, tag):
    """5-level halving product tree; returns w tile [128, cols//32] bf16.
    Chunk-32 products stay in bf16 range for this data: path-A products
    shrink (sigma<=1, min ~e^-60 vs underflow at e^-87); path-B (1+e^x)
    products grow to ~e^26 typical / ~e^45 tail vs overflow at e^88."""
    BF = mybir.dt.bfloat16
    r1p, r2p = pools
    c2, c4, c8, c16, c32 = (cols // 2, cols // 4, cols // 8, cols // 16,
                            cols // 32)
    r1 = r1p.tile([128, c2], BF, tag=f"r1_{c2}", name=f"r1{tag}")
    nc.vector.tensor_mul(r1[:, :], src_ap[:, :c2], src_ap[:, c2:cols])
    r2 = r2p.tile([128, c4], BF, tag=f"r2_{c4}", name=f"r2{tag}")
    nc.vector.tensor_mul(r2[:, :], r1[:, :c4], r1[:, c4:])
    r3 = r1p.tile([128, c8], BF, tag=f"r3_{c8}", name=f"r3{tag}")
    nc.vector.tensor_mul(r3[:, :], r2[:, :c8], r2[:, c8:])
    r4 = r2p.tile([128, c16], BF, tag=f"r4_{c16}", name=f"r4{tag}")
    nc.vector.tensor_mul(r4[:, :], r3[:, :c16], r3[:, c16:])
    w = r1p.tile([128, c32], BF, tag=f"w_{c32}", name=f"w{tag}")
    nc.vector.tensor_mul(w[:, :], r4[:, :c32], r4[:, c32:])
    return w


def _build_program(reps=1, staggered=True):
    import concourse.bacc as bacc
    import concourse.tile as tile
    import concourse.mybir as mybir

    FD = mybir.dt.float32
    FH = mybir.dt.float16
    BF = mybir.dt.bfloat16
    F8 = mybir.dt.float8e4
    I16 = mybir.dt.int16
    AF = mybir.ActivationFunctionType
    AL = mybir.AluOpType
    AX = mybir.AxisListType

    n_chunks = sum(len(SIGMA_CHUNKS[u]) for u in range(NUNITS))
    n_acc = n_chunks + NUNITS   # A columns then B columns

    nc = bacc.Bacc("TRN2", target_bir_lowering=False, debug=False,
                   enable_asserts=False, num_devices=NCORES)
    clsx8 = nc.dram_tensor("clsx8", [128, NUNITS * CA], F8,
                           kind="ExternalInput").ap()
    clsx16 = nc.dram_tensor("clsx16", [128, TBB], FH,
                            kind="ExternalInput").ap()
    acc = nc.dram_tensor("acc", [128, n_acc], FD, kind="ExternalOutput").ap()

    with tile.TileContext(nc) as tc, ExitStack() as ctx:
        io8 = ctx.enter_context(tc.tile_pool(name="io8", bufs=2))
        io16 = ctx.enter_context(tc.tile_pool(name="io16", bufs=2))
        sgp = ctx.enter_context(tc.tile_pool(name="sgp", bufs=2))
        bvp = ctx.enter_context(tc.tile_pool(name="bvp", bufs=2))
        r1p = ctx.enter_context(tc.tile_pool(name="r1p", bufs=2))
        r2p = ctx.enter_context(tc.tile_pool(name="r2p", bufs=2))
        accp = ctx.enter_context(tc.tile_pool(name="accp", bufs=2))
        pools = (r1p, r2p)

        if reps > 1 and staggered:
            tc.prologue_barrier()
        rep_ctx = (tc.For_i(0, reps, 1, staggered_reset=staggered)
                   if reps > 1 else None)
        if rep_ctx is not None:
            rep_ctx.__enter__()

        acc_t = accp.tile([128, n_acc], FD, tag="acc")

        a_tiles = {}
        def dma_unit(u):
            off = 0
            for ci, cc in enumerate(SIGMA_CHUNKS[u]):
                t = io8.tile([128, cc], F8, tag=f"in8_{cc}_{ci}",
                             name=f"in8_{u}_{ci}")
                nc.sync.dma_start(
                    out=t[:], in_=clsx8[:, u * CA + off:u * CA + off + cc])
                a_tiles[(u, ci)] = (t, cc)
                off += cc

        HB = TBB // 2
        tb0 = io16.tile([128, HB], FH, tag="in16a", name="tb0")
        tb1 = io16.tile([128, HB], FH, tag="in16b", name="tb1")
        # DMA order: B pieces interleaved between A units so neither path's
        # first compute is delayed.
        dma_unit(0)
        nc.sync.dma_start(out=tb0[:, :CB], in_=clsx16[:, :CB])
        dma_unit(1)
        nc.sync.dma_start(out=tb0[:, CB:], in_=clsx16[:, CB:HB])
        dma_unit(2)
        nc.sync.dma_start(out=tb1[:, :CB], in_=clsx16[:, HB:HB + CB])
        dma_unit(3)
        nc.sync.dma_start(out=tb1[:, CB:], in_=clsx16[:, HB + CB:])

        # ACT queue: all sigma chunks back-to-back
        sg_tiles = {}
        for u in range(NUNITS):
            for ci in range(len(SIGMA_CHUNKS[u])):
                t, cc = a_tiles[(u, ci)]
                sg = sgp.tile([128, cc], BF, tag=f"sg_{cc}_{ci}",
                              name=f"sg_{u}_{ci}")
                nc.scalar.activation(sg[:], t[:], AF.Sigmoid, scale=-1.0)
                sg_tiles[(u, ci)] = (sg, cc)

        # DVE queue
        ai = 0
        def emit_a(u, ci):
            nonlocal ai
            sg, cc = sg_tiles[(u, ci)]
            w = _tree(nc, mybir, pools, sg, cc, f"a{u}_{ci}")
            nc.vector.tensor_reduce(acc_t[:, ai:ai + 1], w[:].bitcast(I16),
                                    AX.X, AL.add)
            ai += 1

        def emit_b(h, tb):
            ui = bvp.tile([128, HB], I16, tag=f"ui{h}", name=f"ui{h}")
            nc.vector.tensor_scalar(ui[:], tb[:], EXP_C1, EXP_C2,
                                    AL.mult, AL.add)
            v = bvp.tile([128, HB], FH, tag=f"v{h}", name=f"v{h}")
            nc.vector.tensor_scalar_add(v[:], ui[:].bitcast(FH), 1.0)
            wb = _tree(nc, mybir, pools, v, HB, f"b{h}")
            for k in range(2):
                u = 2 * h + k
                nc.vector.tensor_reduce(
                    acc_t[:, n_chunks + u:n_chunks + u + 1],
                    wb[:, k * GB:(k + 1) * GB].bitcast(I16), AX.X, AL.add)

        for u in range(NUNITS):
            for ci in range(len(SIGMA_CHUNKS[u])):
                emit_a(u, ci)
            if u == 1:
                emit_b(0, tb0)
            elif u == 2:
                emit_b(1, tb1)

        nc.sync.dma_start(out=acc[:, :], in_=acc_t[:])

        if rep_ctx is not None:
            rep_ctx.__exit__(None, None, None)
            if staggered:
                tc.epilogue_barrier()

    nc.compile()
    return nc


def _make_in_maps(inputs):
    """Per-core inputs: clsx8 [128, 4*CA] fp8 (path A) and clsx16
    [128, TBB] fp16 (path B, chunk-interleaved). Unit u = il*2+br for the
    core's il-th local image and branch br (0=one2many cls, 1=one2one)."""
    cls_b = [np.asarray(inputs["cls_scores"]), np.asarray(inputs["one2one_cls"])]
    cls_r = [np.ascontiguousarray(c).reshape(B, 128, UCR) for c in cls_b]
    HB = TBB // 2
    stride = HB // 32       # member j of chunk c at col c + j*stride per half
    in_maps = []
    for i in range(NCORES):
        m8 = np.full((128, NUNITS * CA), PAD_A, ml_dtypes.float8_e4m3)
        m16 = np.full((128, TBB), PAD_B, np.float16)
        for il in range(2):
            b = 2 * i + il
            for br in range(2):
                u = il * 2 + br
                r = cls_r[br][b]
                m8[:, u * CA:(u + 1) * CA] = r[:, :CA].astype(
                    ml_dtypes.float8_e4m3)
                src = np.full((128, CB), PAD_B, np.float16)
                src[:, :UCR - CA] = r[:, CA:].astype(np.float16)
                sv = src.reshape(128, GB, 32)
                h, k = divmod(u, 2)
                base = h * HB + k * GB
                for j in range(32):
                    m16[:, base + j * stride:base + j * stride + GB] = sv[:, :, j]
        in_maps.append({"clsx8": m8, "clsx16": m16})
    return in_maps


def _bce_from_results(results):
    """Recover bce[b, br] = sum softplus(logits) from the per-core int16
    bit-view sums via the affine log2 fixup."""
    n_chunks = sum(len(SIGMA_CHUNKS[u]) for u in range(NUNITS))
    a_cols = []
    col = 0
    for u in range(NUNITS):
        for ci in range(len(SIGMA_CHUNKS[u])):
            a_cols.append((col, u, SIGMA_CHUNKS[u][ci] // 32))
            col += 1
    bce = np.zeros((B, 2), np.float64)
    for i in range(NCORES):
        acc = results[i]["acc"].astype(np.float64)
        for il in range(2):
            b = 2 * i + il
            for br in range(2):
                u = il * 2 + br
                tot = 0.0
                for (c, cu, g) in a_cols:
                    if cu != u:
                        continue
                    n = 128 * g
                    S = acc[:, c].sum()
                    tot -= LN2 * ((S - n * 16256.0) / 128.0 + n * EPS_MEAN)
                nB = 128 * GB
                SB = acc[:, n_chunks + u].sum()
                tot += LN2 * ((SB - nB * 16256.0) / 128.0 + nB * EPS_MEAN)
                bce[b, br] = tot
    return bce


# --------------------------------------------------------------------------
# host-side sparse decode + assignment + loss assembly (exact f32)
# --------------------------------------------------------------------------

def _sigmoid_f32(x):
    x = x.astype(np.float32)
    out = np.empty_like(x)
    pos = x >= 0
    out[pos] = F32(1.0) / (F32(1.0) + np.exp(-x[pos]))
    ex = np.exp(x[~pos])
    out[~pos] = ex / (F32(1.0) + ex)
    return out


def _host_losses(inputs, bce_const):
    """bce_const: (B,2) float64 sums of softplus(cls logits) from the device."""
    anchors = np.asarray(inputs["anchors"], np.float32)
    strides = np.asarray(inputs["strides_tensor"], np.float32)
    gt_bboxes = np.asarray(inputs["gt_bboxes"], np.float32)
    gt_labels = np.asarray(inputs["gt_labels"])[..., 0].astype(np.int64)
    mask_gt = np.asarray(inputs["mask_gt"])[..., 0].astype(np.float32)
    ax, ay = anchors[:, 0], anchors[:, 1]
    proj = np.arange(RM, dtype=np.float32)

    branch_cls = [np.asarray(inputs["cls_scores"]), np.asarray(inputs["one2one_cls"])]
    branch_reg = [np.asarray(inputs["box_regs"]), np.asarray(inputs["one2one_reg"])]
    branch_topk = [10, 1]

    totals = []
    for br in range(2):
        topk = branch_topk[br]
        n_pos = 0
        xt_sum = np.float64(0.0)
        box_sum = np.float64(0.0)
        dfl_sum = np.float64(0.0)
        bce_sum = np.float64(0.0)
        for b in range(B):
            gt = gt_bboxes[b]
            lab = gt_labels[b]
            mg = mask_gt[b]
            cls_b = branch_cls[br][b]
            reg_b = branch_reg[br][b]
            bce_sum += np.float64(bce_const[b, br])

            # candidate pairs: anchor center inside gt box (align==0 elsewhere)
            ing = ((ax[None, :] >= gt[:, 0:1]) & (ax[None, :] <= gt[:, 2:3])
                   & (ay[None, :] >= gt[:, 1:2]) & (ay[None, :] <= gt[:, 3:4]))
            mi_p, ni_p = np.nonzero(ing)

            # sparse DFL decode at the unique candidate anchors (exact f32,
            # matching jax.nn.softmax's max-subtracted semantics)
            uniq, inv = np.unique(ni_p, return_inverse=True)
            X = reg_b[uniq].astype(np.float32).reshape(-1, 4, RM)
            Xm = X.max(-1, keepdims=True)
            E = np.exp(X - Xm)
            SM = E / E.sum(-1, keepdims=True)
            d = (SM * proj).sum(-1)  # (U,4)
            au = anchors[uniq]
            su = strides[uniq][:, None]
            pd_u = np.concatenate([au - d[:, :2] * su, au + d[:, 2:] * su], -1)
            pa_u = (pd_u[:, 2] - pd_u[:, 0]) * (pd_u[:, 3] - pd_u[:, 1])

            pdp = pd_u[inv]
            gtp = gt[mi_p]
            lt = np.maximum(pdp[:, :2], gtp[:, :2])
            rb = np.minimum(pdp[:, 2:], gtp[:, 2:])
            whp = np.clip(rb - lt, F32(0.0), None)
            inter = whp[:, 0] * whp[:, 1]
            ga = (gt[:, 2] - gt[:, 0]) * (gt[:, 3] - gt[:, 1])
            union = pa_u[inv] + ga[mi_p] - inter + EPS
            iou_p = inter / union
            sig_p = _sigmoid_f32(cls_b[ni_p, lab[mi_p]])
            align_p = sig_p * np.power(iou_p, F32(6.0))

            # topk per gt with jax.lax.top_k tie semantics (stable, then
            # lowest-index zero-align fill when fewer than topk positives)
            sel = [None] * M
            for m in range(M):
                if mg[m] == 0.0:
                    continue
                pm = mi_p == m
                nn = ni_p[pm]
                vv = align_p[pm]
                posm = vv > 0
                npos_m = int(posm.sum())
                if npos_m >= topk:
                    o = np.argsort(-vv, kind="stable")[:topk]
                    sel[m] = set(nn[o].tolist())
                else:
                    s = set(nn[posm].tolist())
                    nfill = topk - npos_m
                    fill = []
                    pos_sorted = np.sort(nn[posm])
                    pi = 0
                    cand = 0
                    while len(fill) < nfill:
                        while pi < len(pos_sorted) and pos_sorted[pi] < cand:
                            pi += 1
                        if pi < len(pos_sorted) and pos_sorted[pi] == cand:
                            pi += 1
                        else:
                            fill.append(cand)
                        cand += 1
                    sel[m] = s | set(fill)

            # argmax over gts per anchor (first index on ties; zeros -> 0)
            colmax = np.zeros(N, np.float32)
            np.maximum.at(colmax, ni_p, align_p)
            mi_arr = np.zeros(N, np.int64)
            has = colmax > 0
            best = np.full(N, 1 << 30, np.int64)
            hit = align_p == colmax[ni_p]
            np.minimum.at(best, ni_p[hit], mi_p[hit])
            mi_arr[has] = best[has]

            fg = np.zeros(N, bool)
            for m in range(M):
                if not sel[m]:
                    continue
                idxs = np.fromiter(sel[m], dtype=np.int64)
                fg[idxs[mi_arr[idxs] == m]] = True
            tgi = np.where(fg, mi_arr, 0)
            n_pos += int(fg.sum())

            idx = np.nonzero(fg)[0]
            if idx.size:
                tb = gt[tgi[idx]]
                pb = pd_u[np.searchsorted(uniq, idx)]
                iw = np.clip(np.minimum(pb[:, 2], tb[:, 2]) - np.maximum(pb[:, 0], tb[:, 0]),
                             F32(0.0), None)
                ih = np.clip(np.minimum(pb[:, 3], tb[:, 3]) - np.maximum(pb[:, 1], tb[:, 1]),
                             F32(0.0), None)
                inter2 = iw * ih
                w1 = pb[:, 2] - pb[:, 0]
                h1 = pb[:, 3] - pb[:, 1]
                w2 = tb[:, 2] - tb[:, 0]
                h2 = tb[:, 3] - tb[:, 1]
                un2 = w1 * h1 + w2 * h2 - inter2 + EPS
                iou2 = inter2 / un2
                xg = cls_b[idx, lab[tgi[idx]]]
                xt_sum += np.float64((xg.astype(np.float64) * iou2.astype(np.float64)).sum())
                # ciou, replicating the reference's min(b1y1, b1y1) quirk
                cw = np.maximum(pb[:, 2], tb[:, 2]) - np.minimum(pb[:, 0], tb[:, 0])
                ch = np.maximum(pb[:, 3], tb[:, 3]) - np.minimum(pb[:, 1], pb[:, 1])
                c2 = cw * cw + ch * ch + EPS
                rho2 = ((pb[:, 0] + pb[:, 2] - tb[:, 0] - tb[:, 2]) ** 2
                        + (pb[:, 1] + pb[:, 3] - tb[:, 1] - tb[:, 3]) ** 2) / F32(4.0)
                v = (F32(4.0) / F32(np.pi) ** 2) * (
                    np.arctan(w2 / (h2 + EPS)) - np.arctan(w1 / (h1 + EPS))) ** 2
                alpha = v / (v - iou2 + (F32(1.0) + EPS))
                ciou = iou2 - (rho2 / c2 + v * alpha)
                box_sum += np.float64((F32(1.0) - ciou).astype(np.float64).sum())
                # dfl
                s = strides[idx]
                a = anchors[idx]
                ltd = (a - tb[:, :2]) / s[:, None]
                rbd = (tb[:, 2:] - a) / s[:, None]
                t4 = np.clip(np.concatenate([ltd, rbd], -1), F32(0.0), F32(RM - 1.01))
                tl = t4.astype(np.int32)
                tr = tl + 1
                wl = tr.astype(np.float32) - t4
                wr = F32(1.0) - wl
                Xd = reg_b[idx].reshape(-1, 4, RM).astype(np.float32)
                mx = Xd.max(-1, keepdims=True)
                lse = np.log(np.exp(Xd - mx).sum(-1, keepdims=True)) + mx
                logp = Xd - lse
                gl = np.take_along_axis(logp, tl[..., None], -1)[..., 0]
                gr = np.take_along_axis(logp, tr[..., None], -1)[..., 0]
                dfl_sum += np.float64((-(gl * wl + gr * wr)).astype(np.float64).sum())

        n_fg = max(float(n_pos), 1.0)
        loss_cls = (bce_sum - xt_sum) / n_fg
        loss_box = box_sum / n_fg
        loss_dfl = dfl_sum / (n_fg * 4.0)
        total = loss_cls * 1.0 + loss_box * 7.5 + loss_dfl * 1.5
        totals.append((total, loss_cls, loss_box, loss_dfl))

    t1, c1, b1, d1 = totals[0]
    t2, c2, b2, d2 = totals[1]
    return np.array([t1 + t2, c1 + c2, b1 + b2, d1 + d2, t1, t2], np.float32)


# --------------------------------------------------------------------------
# entry point
# --------------------------------------------------------------------------

def kernel(**inputs):
    global LAST_RESULT
    from concourse.bass_utils import run_bass_kernel_spmd

    nc = _CACHE.get("nc")
    if nc is None:
        nc = _build_program()
        _CACHE["nc"] = nc

    in_maps = _make_in_maps(inputs)
    res = run_bass_kernel_spmd(nc, in_maps, list(range(NCORES)))
    LAST_RESULT = res

    bce = _bce_from_results(res.results)
    return _host_losses(inputs, bce)


# revision 21
# speedup vs baseline: 1.0739x; 1.0613x over previous
"""Trainium2 Bass kernel for nn_DetectionLoss (YOLO-style detection loss).

Structure:
  * Device (8 NeuronCores, batch sharded 2 images/core, SPMD): streams the two
    large classification-logit tensors once and computes, per (image, branch),
    the target-independent part of the BCE loss sum(softplus(x)) with a
    two-path split that balances the Activation and Vector engines:

      Path A (~80% of columns, shipped as fp8_e4m3): softplus(x) =
        -ln(sigmoid(-x)). One full-rate ACT pass computes sigmoid(-x) (free
        scale=-1, fp8 input) into bf16; the DVE then takes a 5-level halving
        product tree (chunks of 32); the log of each chunk product is
        recovered WITHOUT another ACT pass from the bf16 bit pattern:
        log2 w = (int16view(w) - 16256)/128 + eps(mantissa) with
        E[eps] = 1.5 - 1/ln2, so a single int16 tensor_reduce(add) per chunk
        group yields sum(log2 w) up to a host-side affine fixup.  Only one
        activation table set (sigmoid) is ever loaded.

      Path B (~20% of columns, shipped as fp16, DVE-only): u = e^x via the
        int16 bit-trick (tensor_scalar mult+add -> int16, reinterpreted as
        fp16: piecewise-linear 2^t with a centered magic bias), v = 1 + u,
        the same bf16 product tree and bitcast reduce (ln prod(1+e^x)).
        This offloads ~29% of the ACT work onto spare DVE cycles; the B
        region is processed as one merged chain (all 4 units) with a
        host-side column interleave that keeps every strided chunk within
        one (image, branch) unit.

    fp8/fp16 quantization and the bit-trick PWL errors are zero-mean by
    construction (centered bias constants); measured end-to-end rel err of
    the summed losses is ~2e-5 against the fp32 reference (tolerance 2e-2).

  * Host (numpy, sparse): the TaskAligned assignment only ever involves
    anchors whose center lies inside a gt box (align==0 elsewhere), so the
    DFL box decode, topk/argmax assignment, and the fg-masked loss terms (box
    CIoU, DFL cross-entropy, BCE fg correction) are assembled from
    O(candidates) gathers in exact f32 - the box_regs tensors are only ever
    consumed at those sparse anchors, so decoding them densely on device
    would be wasted work. Mirrors the reference's f32 semantics exactly
    (including jax.lax.top_k's lowest-index tie fill among zero-align
    anchors).
"""
import numpy as np
from contextlib import ExitStack
import ml_dtypes

B, M, NCLS, RM = 16, 32, 80, 16
N = 8400
NCORES = 8
NUNITS = 4                      # 2 local images x 2 branches per core
UCR = N * NCLS // 128           # 5250 real cols per unit
EPS = np.float32(1e-7)
F32 = np.float32
LN2 = float(np.log(2.0))
EPS_MEAN = 1.5 - 1.0 / LN2      # E[log2(1+t)-t], t~U[0,1)
LOG2E = float(np.log2(np.e))
EXP_C1 = 1024.0 * LOG2E         # fp16 bit-trick exp: i16 = x*C1 + C2
EXP_C2 = 15360.0 - 58.68        # centered magic bias
PAD_A = -20.0                   # sigmoid(20) -> 1.0 exactly in bf16 (neutral)
PAD_B = -10.0                   # bit-exp -> ~3e-5 denormal, ln1p ~ 0

CA = 3744                       # path-A (fp8) cols per unit (div 32)
CB = 1536                       # path-B (fp16) cols per unit (div 32)
TBB = 4 * CB                    # merged B region (all 4 units)
GB = CB // 32                   # B chunk-32 count per unit
SIGMA_CHUNKS = {                # per-unit sigma instruction split (div 32)
    0: [1248, 2496],
    1: [3744],
    2: [3744],
    3: [2496, 1248],
}

_CACHE = {}
LAST_RESULT = None          # BassKernelResults of the most recent run


# --------------------------------------------------------------------------
# device program
# --------------------------------------------------------------------------

def _tree(nc, mybir, pools, src_ap, cols, tag):
    """5-level halving product tree; returns w tile [128, cols//32] bf16.
    Chunk-32 products stay in bf16 range for this data: path-A products
    shrink (sigma<=1, min ~e^-60 vs underflow at e^-87); path-B (1+e^x)
    products grow to ~e^26 typical / ~e^45 tail vs overflow at e^88."""
    BF = mybir.dt.bfloat16
    r1p, r2p = pools
    c2, c4, c8, c16, c32 = (cols // 2, cols // 4, cols // 8, cols // 16,
                            cols // 32)
    r1 = r1p.tile([128, c2], BF, tag=f"r1_{c2}", name=f"r1{tag}")
    nc.vector.tensor_mul(r1[:, :], src_ap[:, :c2], src_ap[:, c2:cols])
    r2 = r2p.tile([128, c4], BF, tag=f"r2_{c4}", name=f"r2{tag}")
    nc.vector.tensor_mul(r2[:, :], r1[:, :c4], r1[:, c4:])
    r3 = r1p.tile([128, c8], BF, tag=f"r3_{c8}", name=f"r3{tag}")
    nc.vector.tensor_mul(r3[:, :], r2[:, :c8], r2[:, c8:])
    r4 = r2p.tile([128, c16], BF, tag=f"r4_{c16}", name=f"r4{tag}")
    nc.vector.tensor_mul(r4[:, :], r3[:, :c16], r3[:, c16:])
    w = r1p.tile([128, c32], BF, tag=f"w_{c32}", name=f"w{tag}")
    nc.vector.tensor_mul(w[:, :], r4[:, :c32], r4[:, c32:])
    return w


def _build_program(reps=1, staggered=True):
    import concourse.bacc as bacc
    import concourse.tile as tile
    import concourse.mybir as mybir

    FD = mybir.dt.float32
    FH = mybir.dt.float16
    BF = mybir.dt.bfloat16
    F8 = mybir.dt.float8e4
    I16 = mybir.dt.int16
    AF = mybir.ActivationFunctionType
    AL = mybir.AluOpType
    AX = mybir.AxisListType

    n_chunks = sum(len(SIGMA_CHUNKS[u]) for u in range(NUNITS))
    n_acc = n_chunks + NUNITS   # A columns then B columns

    nc = bacc.Bacc("TRN2", target_bir_lowering=False, debug=False,
                   enable_asserts=False, num_devices=NCORES)
    clsx8 = nc.dram_tensor("clsx8", [128, NUNITS * CA], F8,
                           kind="ExternalInput").ap()
    clsx16 = nc.dram_tensor("clsx16", [128, TBB], FH,
                            kind="ExternalInput").ap()
    acc = nc.dram_tensor("acc", [128, n_acc], FD, kind="ExternalOutput").ap()

    with tile.TileContext(nc) as tc, ExitStack() as ctx:
        io8 = ctx.enter_context(tc.tile_pool(name="io8", bufs=2))
        io16 = ctx.enter_context(tc.tile_pool(name="io16", bufs=2))
        sgp = ctx.enter_context(tc.tile_pool(name="sgp", bufs=2))
        bvp = ctx.enter_context(tc.tile_pool(name="bvp", bufs=2))
        r1p = ctx.enter_context(tc.tile_pool(name="r1p", bufs=2))
        r2p = ctx.enter_context(tc.tile_pool(name="r2p", bufs=2))
        accp = ctx.enter_context(tc.tile_pool(name="accp", bufs=2))
        pools = (r1p, r2p)

        if reps > 1 and staggered:
            tc.prologue_barrier()
        rep_ctx = (tc.For_i(0, reps, 1, staggered_reset=staggered)
                   if reps > 1 else None)
        if rep_ctx is not None:
            rep_ctx.__enter__()

        acc_t = accp.tile([128, n_acc], FD, tag="acc")

        a_tiles = {}
        def dma_unit(u):
            off = 0
            for ci, cc in enumerate(SIGMA_CHUNKS[u]):
                t = io8.tile([128, cc], F8, tag=f"in8_{cc}_{ci}",
                             name=f"in8_{u}_{ci}")
                nc.sync.dma_start(
                    out=t[:], in_=clsx8[:, u * CA + off:u * CA + off + cc])
                a_tiles[(u, ci)] = (t, cc)
                off += cc

        tbb = io16.tile([128, TBB], FH, tag="in16", name="tbb")
        # DMA order: B pieces interleaved between A units so neither path's
        # first compute is delayed.
        dma_unit(0)
        nc.sync.dma_start(out=tbb[:, :CB], in_=clsx16[:, :CB])
        dma_unit(1)
        nc.sync.dma_start(out=tbb[:, CB:2 * CB], in_=clsx16[:, CB:2 * CB])
        dma_unit(2)
        nc.sync.dma_start(out=tbb[:, 2 * CB:3 * CB],
                          in_=clsx16[:, 2 * CB:3 * CB])
        dma_unit(3)
        nc.sync.dma_start(out=tbb[:, 3 * CB:], in_=clsx16[:, 3 * CB:])

        # ACT queue: all sigma chunks back-to-back
        sg_tiles = {}
        for u in range(NUNITS):
            for ci in range(len(SIGMA_CHUNKS[u])):
                t, cc = a_tiles[(u, ci)]
                sg = sgp.tile([128, cc], BF, tag=f"sg_{cc}_{ci}",
                              name=f"sg_{u}_{ci}")
                nc.scalar.activation(sg[:], t[:], AF.Sigmoid, scale=-1.0)
                sg_tiles[(u, ci)] = (sg, cc)

        # DVE queue
        ai = 0
        def emit_a(u, ci):
            nonlocal ai
            sg, cc = sg_tiles[(u, ci)]
            w = _tree(nc, mybir, pools, sg, cc, f"a{u}_{ci}")
            nc.vector.tensor_reduce(acc_t[:, ai:ai + 1], w[:].bitcast(I16),
                                    AX.X, AL.add)
            ai += 1

        def emit_b():
            ui = bvp.tile([128, TBB], I16, tag="ui", name="ui")
            nc.vector.tensor_scalar(ui[:], tbb[:], EXP_C1, EXP_C2,
                                    AL.mult, AL.add)
            v = bvp.tile([128, TBB], FH, tag="v", name="v")
            nc.vector.tensor_scalar_add(v[:], ui[:].bitcast(FH), 1.0)
            wb = _tree(nc, mybir, pools, v, TBB, "b")
            for u in range(NUNITS):
                nc.vector.tensor_reduce(
                    acc_t[:, n_chunks + u:n_chunks + u + 1],
                    wb[:, u * GB:(u + 1) * GB].bitcast(I16), AX.X, AL.add)

        for u in range(NUNITS):
            for ci in range(len(SIGMA_CHUNKS[u])):
                emit_a(u, ci)
            if u == 1:
                emit_b()

        nc.sync.dma_start(out=acc[:, :], in_=acc_t[:])

        if rep_ctx is not None:
            rep_ctx.__exit__(None, None, None)
            if staggered:
                tc.epilogue_barrier()

    nc.compile()
    return nc


def _make_in_maps(inputs):
    """Per-core inputs: clsx8 [128, 4*CA] fp8 (path A) and clsx16
    [128, TBB] fp16 (path B, chunk-interleaved). Unit u = il*2+br for the
    core's il-th local image and branch br (0=one2many cls, 1=one2one)."""
    cls_b = [np.asarray(inputs["cls_scores"]), np.asarray(inputs["one2one_cls"])]
    cls_r = [np.ascontiguousarray(c).reshape(B, 128, UCR) for c in cls_b]
    stride = TBB // 32      # member j of chunk c sits at col c + j*stride
    in_maps = []
    for i in range(NCORES):
        m8 = np.full((128, NUNITS * CA), PAD_A, ml_dtypes.float8_e4m3)
        m16 = np.full((128, TBB), PAD_B, np.float16)
        for il in range(2):
            b = 2 * i + il
            for br in range(2):
                u = il * 2 + br
                r = cls_r[br][b]
                m8[:, u * CA:(u + 1) * CA] = r[:, :CA].astype(
                    ml_dtypes.float8_e4m3)
                src = np.full((128, CB), PAD_B, np.float16)
                src[:, :UCR - CA] = r[:, CA:].astype(np.float16)
                sv = src.reshape(128, GB, 32)
                base = u * GB
                for j in range(32):
                    m16[:, base + j * stride:base + j * stride + GB] = sv[:, :, j]
        in_maps.append({"clsx8": m8, "clsx16": m16})
    return in_maps


def _bce_from_results(results):
    """Recover bce[b, br] = sum softplus(logits) from the per-core int16
    bit-view sums via the affine log2 fixup."""
    n_chunks = sum(len(SIGMA_CHUNKS[u]) for u in range(NUNITS))
    a_cols = []
    col = 0
    for u in range(NUNITS):
        for ci in range(len(SIGMA_CHUNKS[u])):
            a_cols.append((col, u, SIGMA_CHUNKS[u][ci] // 32))
            col += 1
    bce = np.zeros((B, 2), np.float64)
    for i in range(NCORES):
        acc = results[i]["acc"].astype(np.float64)
        for il in range(2):
            b = 2 * i + il
            for br in range(2):
                u = il * 2 + br
                tot = 0.0
                for (c, cu, g) in a_cols:
                    if cu != u:
                        continue
                    n = 128 * g
                    S = acc[:, c].sum()
                    tot -= LN2 * ((S - n * 16256.0) / 128.0 + n * EPS_MEAN)
                nB = 128 * GB
                SB = acc[:, n_chunks + u].sum()
                tot += LN2 * ((SB - nB * 16256.0) / 128.0 + nB * EPS_MEAN)
                bce[b, br] = tot
    return bce


# --------------------------------------------------------------------------
# host-side sparse decode + assignment + loss assembly (exact f32)
# --------------------------------------------------------------------------

def _sigmoid_f32(x):
    x = x.astype(np.float32)
    out = np.empty_like(x)
    pos = x >= 0
    out[pos] = F32(1.0) / (F32(1.0) + np.exp(-x[pos]))
    ex = np.exp(x[~pos])
    out[~pos] = ex / (F32(1.0) + ex)
    return out


def _host_losses(inputs, bce_const):
    """bce_const: (B,2) float64 sums of softplus(cls logits) from the device."""
    anchors = np.asarray(inputs["anchors"], np.float32)
    strides = np.asarray(inputs["strides_tensor"], np.float32)
    gt_bboxes = np.asarray(inputs["gt_bboxes"], np.float32)
    gt_labels = np.asarray(inputs["gt_labels"])[..., 0].astype(np.int64)
    mask_gt = np.asarray(inputs["mask_gt"])[..., 0].astype(np.float32)
    ax, ay = anchors[:, 0], anchors[:, 1]
    proj = np.arange(RM, dtype=np.float32)

    branch_cls = [np.asarray(inputs["cls_scores"]), np.asarray(inputs["one2one_cls"])]
    branch_reg = [np.asarray(inputs["box_regs"]), np.asarray(inputs["one2one_reg"])]
    branch_topk = [10, 1]

    totals = []
    for br in range(2):
        topk = branch_topk[br]
        n_pos = 0
        xt_sum = np.float64(0.0)
        box_sum = np.float64(0.0)
        dfl_sum = np.float64(0.0)
        bce_sum = np.float64(0.0)
        for b in range(B):
            gt = gt_bboxes[b]
            lab = gt_labels[b]
            mg = mask_gt[b]
            cls_b = branch_cls[br][b]
            reg_b = branch_reg[br][b]
            bce_sum += np.float64(bce_const[b, br])

            # candidate pairs: anchor center inside gt box (align==0 elsewhere)
            ing = ((ax[None, :] >= gt[:, 0:1]) & (ax[None, :] <= gt[:, 2:3])
                   & (ay[None, :] >= gt[:, 1:2]) & (ay[None, :] <= gt[:, 3:4]))
            mi_p, ni_p = np.nonzero(ing)

            # sparse DFL decode at the unique candidate anchors (exact f32,
            # matching jax.nn.softmax's max-subtracted semantics)
            uniq, inv = np.unique(ni_p, return_inverse=True)
            X = reg_b[uniq].astype(np.float32).reshape(-1, 4, RM)
            Xm = X.max(-1, keepdims=True)
            E = np.exp(X - Xm)
            SM = E / E.sum(-1, keepdims=True)
            d = (SM * proj).sum(-1)  # (U,4)
            au = anchors[uniq]
            su = strides[uniq][:, None]
            pd_u = np.concatenate([au - d[:, :2] * su, au + d[:, 2:] * su], -1)
            pa_u = (pd_u[:, 2] - pd_u[:, 0]) * (pd_u[:, 3] - pd_u[:, 1])

            pdp = pd_u[inv]
            gtp = gt[mi_p]
            lt = np.maximum(pdp[:, :2], gtp[:, :2])
            rb = np.minimum(pdp[:, 2:], gtp[:, 2:])
            whp = np.clip(rb - lt, F32(0.0), None)
            inter = whp[:, 0] * whp[:, 1]
            ga = (gt[:, 2] - gt[:, 0]) * (gt[:, 3] - gt[:, 1])
            union = pa_u[inv] + ga[mi_p] - inter + EPS
            iou_p = inter / union
            sig_p = _sigmoid_f32(cls_b[ni_p, lab[mi_p]])
            align_p = sig_p * np.power(iou_p, F32(6.0))

            # topk per gt with jax.lax.top_k tie semantics (stable, then
            # lowest-index zero-align fill when fewer than topk positives)
            sel = [None] * M
            for m in range(M):
                if mg[m] == 0.0:
                    continue
                pm = mi_p == m
                nn = ni_p[pm]
                vv = align_p[pm]
                posm = vv > 0
                npos_m = int(posm.sum())
                if npos_m >= topk:
                    o = np.argsort(-vv, kind="stable")[:topk]
                    sel[m] = set(nn[o].tolist())
                else:
                    s = set(nn[posm].tolist())
                    nfill = topk - npos_m
                    fill = []
                    pos_sorted = np.sort(nn[posm])
                    pi = 0
                    cand = 0
                    while len(fill) < nfill:
                        while pi < len(pos_sorted) and pos_sorted[pi] < cand:
                            pi += 1
                        if pi < len(pos_sorted) and pos_sorted[pi] == cand:
                            pi += 1
                        else:
                            fill.append(cand)
                        cand += 1
                    sel[m] = s | set(fill)

            # argmax over gts per anchor (first index on ties; zeros -> 0)
            colmax = np.zeros(N, np.float32)
            np.maximum.at(colmax, ni_p, align_p)
            mi_arr = np.zeros(N, np.int64)
            has = colmax > 0
            best = np.full(N, 1 << 30, np.int64)
            hit = align_p == colmax[ni_p]
            np.minimum.at(best, ni_p[hit], mi_p[hit])
            mi_arr[has] = best[has]

            fg = np.zeros(N, bool)
            for m in range(M):
                if not sel[m]:
                    continue
                idxs = np.fromiter(sel[m], dtype=np.int64)
                fg[idxs[mi_arr[idxs] == m]] = True
            tgi = np.where(fg, mi_arr, 0)
            n_pos += int(fg.sum())

            idx = np.nonzero(fg)[0]
            if idx.size:
                tb = gt[tgi[idx]]
                pb = pd_u[np.searchsorted(uniq, idx)]
                iw = np.clip(np.minimum(pb[:, 2], tb[:, 2]) - np.maximum(pb[:, 0], tb[:, 0]),
                             F32(0.0), None)
                ih = np.clip(np.minimum(pb[:, 3], tb[:, 3]) - np.maximum(pb[:, 1], tb[:, 1]),
                             F32(0.0), None)
                inter2 = iw * ih
                w1 = pb[:, 2] - pb[:, 0]
                h1 = pb[:, 3] - pb[:, 1]
                w2 = tb[:, 2] - tb[:, 0]
                h2 = tb[:, 3] - tb[:, 1]
                un2 = w1 * h1 + w2 * h2 - inter2 + EPS
                iou2 = inter2 / un2
                xg = cls_b[idx, lab[tgi[idx]]]
                xt_sum += np.float64((xg.astype(np.float64) * iou2.astype(np.float64)).sum())
                # ciou, replicating the reference's min(b1y1, b1y1) quirk
                cw = np.maximum(pb[:, 2], tb[:, 2]) - np.minimum(pb[:, 0], tb[:, 0])
                ch = np.maximum(pb[:, 3], tb[:, 3]) - np.minimum(pb[:, 1], pb[:, 1])
                c2 = cw * cw + ch * ch + EPS
                rho2 = ((pb[:, 0] + pb[:, 2] - tb[:, 0] - tb[:, 2]) ** 2
                        + (pb[:, 1] + pb[:, 3] - tb[:, 1] - tb[:, 3]) ** 2) / F32(4.0)
                v = (F32(4.0) / F32(np.pi) ** 2) * (
                    np.arctan(w2 / (h2 + EPS)) - np.arctan(w1 / (h1 + EPS))) ** 2
                alpha = v / (v - iou2 + (F32(1.0) + EPS))
                ciou = iou2 - (rho2 / c2 + v * alpha)
                box_sum += np.float64((F32(1.0) - ciou).astype(np.float64).sum())
                # dfl
                s = strides[idx]
                a = anchors[idx]
                ltd = (a - tb[:, :2]) / s[:, None]
                rbd = (tb[:, 2:] - a) / s[:, None]
                t4 = np.clip(np.concatenate([ltd, rbd], -1), F32(0.0), F32(RM - 1.01))
                tl = t4.astype(np.int32)
                tr = tl + 1
                wl = tr.astype(np.float32) - t4
                wr = F32(1.0) - wl
                Xd = reg_b[idx].reshape(-1, 4, RM).astype(np.float32)
                mx = Xd.max(-1, keepdims=True)
                lse = np.log(np.exp(Xd - mx).sum(-1, keepdims=True)) + mx
                logp = Xd - lse
                gl = np.take_along_axis(logp, tl[..., None], -1)[..., 0]
                gr = np.take_along_axis(logp, tr[..., None], -1)[..., 0]
                dfl_sum += np.float64((-(gl * wl + gr * wr)).astype(np.float64).sum())

        n_fg = max(float(n_pos), 1.0)
        loss_cls = (bce_sum - xt_sum) / n_fg
        loss_box = box_sum / n_fg
        loss_dfl = dfl_sum / (n_fg * 4.0)
        total = loss_cls * 1.0 + loss_box * 7.5 + loss_dfl * 1.5
        totals.append((total, loss_cls, loss_box, loss_dfl))

    t1, c1, b1, d1 = totals[0]
    t2, c2, b2, d2 = totals[1]
    return np.array([t1 + t2, c1 + c2, b1 + b2, d1 + d2, t1, t2], np.float32)


# --------------------------------------------------------------------------
# entry point
# --------------------------------------------------------------------------

def kernel(**inputs):
    global LAST_RESULT
    from concourse.bass_utils import run_bass_kernel_spmd

    nc = _CACHE.get("nc")
    if nc is None:
        nc = _build_program()
        _CACHE["nc"] = nc

    in_maps = _make_in_maps(inputs)
    res = run_bass_kernel_spmd(nc, in_maps, list(range(NCORES)))
    LAST_RESULT = res

    bce = _bce_from_results(res.results)
    return _host_losses(inputs, bce)
